# revision 20
# baseline (speedup 1.0000x reference)
"""MiniMax-M2 sparse MoE block on 8 Trainium2 NeuronCores.

Strategy: expert-parallel. Core c owns expert c's weights. The host computes
the routing (top-2 of 8, sigmoid scores + bias for selection) in float64,
gathers each expert's tokens, and ships them pre-transposed; each core runs
the gated FFN (silu(x@w1.T) * (x@w3.T)) @ w2.T over its gathered tokens in
float32r (full-rate fp32 matmul mode) and also computes the router logits for
its 1/8 slice of tokens. The host scatter-adds the weighted expert outputs.

Shapes (fixed by the problem): B=4, S=4096, H=2048, I=1024, E=8, top_k=2.

The intermediate dim I is processed in two halves so that one half's weights
(w1/w3 halves [H,512] + w2 half [512,H], fp32) stay resident in SBUF; each
half emits a partial Y (summed on the host). The router block is emitted
between the two halves so its matmuls run on a warm PE and overlap the
half-1 weight reload.
"""
import numpy as np

import concourse.bass as bass
import concourse.mybir as mybir
from concourse import bacc
import concourse.tile as tile
from concourse.bass_utils import run_bass_kernel_spmd

H = 2048
I = 1024
E = 8
TOP_K = 2
KT_H = H // 128          # 16 k-tiles over H
IH = I // 2              # 512: half of intermediate dim
ISUB = IH // 128         # 4 psum subtiles per half
KT_I = IH // 128         # 4 k-tiles over I-half
NTOK = 512               # full token tile (matmul moving dim)
TAIL = 256               # token padding granularity (f32r full rate needs >=256)
f32 = mybir.dt.float32
f32r = mybir.dt.float32r
ACT_FN = mybir.ActivationFunctionType.Silu  # test_sim overrides (CoreSim lacks Silu)


def _ensure_ntff_hook():
    """If the environment requests tracing (BASS_TRACE) but lacks the
    antenv.axon_hooks module, inject it so profiling works instead of being
    silently skipped."""
    try:
        from antenv.axon_hooks import get_axon_ntff_profile_hook  # noqa: F401
        return
    except ImportError:
        pass
    try:
        import sys, types
        import antenv
        from trn_agent_boot.trn_boot import _ntff_profile_via_ctypes
        mod = types.ModuleType("antenv.axon_hooks")
        mod._hook = _ntff_profile_via_ctypes("/opt/axon/libaxon_pjrt.so")
        mod.set_axon_ntff_profile_hook = lambda h: setattr(mod, "_hook", h)
        mod.get_axon_ntff_profile_hook = lambda: mod._hook
        sys.modules["antenv.axon_hooks"] = mod
        antenv.axon_hooks = mod
    except Exception:
        pass


_BUILD_CACHE = {}
LAST_RESULT = None


def _tiles(n, tail_last):
    widths = [NTOK] * (n // NTOK)
    if n % NTOK:
        widths = widths + [TAIL] if tail_last else [TAIL] + widths
    offs = np.cumsum([0] + widths)[:-1].tolist()
    return widths, offs


def build(k1, k2, tslice):
    """Build the SPMD program: each core processes two expert slots with
    K1 / K2 gathered tokens (multiples of TAIL) and a router slice of
    `tslice` tokens."""
    key = (k1, k2, tslice)
    if key in _BUILD_CACHE:
        return _BUILD_CACHE[key]
    assert k1 % TAIL == 0 and k2 % TAIL == 0 and tslice % NTOK == 0
    ns_r = tslice // NTOK
    R = k1 + k2

    nc = bacc.Bacc("TRN2", target_bir_lowering=False, debug=False, num_devices=8)
    xtb = nc.dram_tensor("xtb", [H, R], f32r, kind="ExternalInput").ap()
    cwb = nc.dram_tensor("cwb", [128, R], f32, kind="ExternalInput").ap()
    w1tb = nc.dram_tensor("w1tb", [2, 2, KT_H, 128, IH], f32r, kind="ExternalInput").ap()
    w3tb = nc.dram_tensor("w3tb", [2, 2, KT_H, 128, IH], f32r, kind="ExternalInput").ap()
    w2tb = nc.dram_tensor("w2tb", [2, 2, KT_I, 128, H], f32r, kind="ExternalInput").ap()
    xrb = nc.dram_tensor("xrb", [ns_r, KT_H, 128, NTOK], f32r, kind="ExternalInput").ap()
    gwtb = nc.dram_tensor("gwtb", [KT_H, 128, E], f32r, kind="ExternalInput").ap()
    yb = nc.dram_tensor("yb", [2, H, R], f32, kind="ExternalOutput").ap()
    logb = nc.dram_tensor("logb", [E, tslice], f32, kind="ExternalOutput").ap()

    xtb_k = xtb.rearrange("(kt p) r -> kt p r", p=128)

    with tile.TileContext(nc) as tc:
        with (
            tc.tile_pool(name="wp", bufs=1) as wp,
            tc.tile_pool(name="xp", bufs=2) as xp,
            tc.tile_pool(name="pp", bufs=2) as pp,
            tc.tile_pool(name="cp", bufs=2) as cp,
            tc.tile_pool(name="yp", bufs=4) as yp,
            tc.tile_pool(name="rp", bufs=2) as rp,
            tc.tile_pool(name="gups", bufs=5, space="PSUM") as gups,
            tc.tile_pool(name="yps", bufs=2, space="PSUM") as yps,
            tc.tile_pool(name="lps", bufs=1, space="PSUM") as lps,
        ):
            def load_tile_inputs(ph, ti, off, w):
                xt = xp.tile([128, KT_H * w], f32r, tag="x", name=f"x_{ph}_{ti}")
                for ki in range(KT_H):
                    nc.sync.dma_start(
                        out=xt[:, ki * w:(ki + 1) * w],
                        in_=xtb_k[ki][:, off:off + w],
                    )
                cw = cp.tile([128, w], f32, tag="cw", bufs=1, name=f"cw_{ph}_{ti}")
                nc.sync.dma_start(out=cw[:], in_=cwb[:, off:off + w])
                return xt, cw

            def ffn_phase(ph, slot, h, base, widths, offs):
                # tile-0 inputs (and the first router slice) are queued BEFORE
                # the 12.6MB weight block so PE work starts within a few us.
                tile0 = load_tile_inputs(ph, 0, base + offs[0], widths[0])

                w1t, w3t, w2t = [], [], []
                for ki in range(KT_H):
                    t1 = wp.tile([128, IH], f32r, tag=f"w1_{ki}", name=f"w1_{ph}_{ki}")
                    nc.sync.dma_start(out=t1[:], in_=w1tb[slot, h, ki])
                    w1t.append(t1)
                    t3 = wp.tile([128, IH], f32r, tag=f"w3_{ki}", name=f"w3_{ph}_{ki}")
                    nc.sync.dma_start(out=t3[:], in_=w3tb[slot, h, ki])
                    w3t.append(t3)
                for ki in range(KT_I):
                    t2 = wp.tile([128, H], f32r, tag=f"w2_{ki}", name=f"w2_{ph}_{ki}")
                    nc.sync.dma_start(out=t2[:], in_=w2tb[slot, h, ki])
                    w2t.append(t2)

                for ti, (off0, w) in enumerate(zip(offs, widths)):
                    off = base + off0
                    if ti >= 1 and router_pending:
                        router_pending.pop(0)()
                    if ti == 0:
                        xt, cw = tile0
                    else:
                        xt, cw = load_tile_inputs(ph, ti, off, w)

                    pt = pp.tile([128, ISUB * w], f32r, tag="p", name=f"p_{ph}_{ti}")
                    for isub in range(ISUB):
                        gp = gups.tile([128, w], f32, tag="gu", name=f"g_{ph}_{ti}_{isub}")
                        up = gups.tile([128, w], f32, tag="gu", name=f"u_{ph}_{ti}_{isub}")
                        msl = slice(isub * 128, (isub + 1) * 128)
                        for ki in range(KT_H):
                            nc.tensor.matmul(
                                gp[:],
                                w1t[ki][:, msl],
                                xt[:, ki * w:(ki + 1) * w],
                                start=(ki == 0),
                                stop=(ki == KT_H - 1),
                            )
                        for ki in range(KT_H):
                            nc.tensor.matmul(
                                up[:],
                                w3t[ki][:, msl],
                                xt[:, ki * w:(ki + 1) * w],
                                start=(ki == 0),
                                stop=(ki == KT_H - 1),
                            )
                        psl = slice(isub * w, (isub + 1) * w)
                        sg = yp.tile([128, w], f32, tag="sg", bufs=1, name=f"sg_{ph}_{ti}_{isub}")
                        nc.scalar.activation(sg[:], gp[:], ACT_FN)
                        nc.vector.tensor_mul(pt[:, psl], up[:], sg[:])
                    for m in range(KT_H):
                        yps_t = yps.tile([128, w], f32, tag="y", name=f"y_{ph}_{ti}_{m}")
                        for ki in range(KT_I):
                            nc.tensor.matmul(
                                yps_t[:],
                                w2t[ki][:, m * 128:(m + 1) * 128],
                                pt[:, ki * w:(ki + 1) * w],
                                start=(ki == 0),
                                stop=(ki == KT_I - 1),
                            )
                        ysb = yp.tile([128, w], f32, tag="ysb", bufs=3, name=f"ysb_{ph}_{ti}_{m}")
                        nc.vector.tensor_mul(ysb[:], yps_t[:], cw[:])  # cw: per-token scale
                        nc.sync.dma_start(
                            out=yb[h][m * 128:(m + 1) * 128, off:off + w], in_=ysb[:]
                        )

            gwt_box = []

            def router_slice(s):
                def emit():
                    if not gwt_box:
                        g = rp.tile([128, KT_H * E], f32r, tag="gwt", bufs=1, name="gwt")
                        for ki in range(KT_H):
                            nc.sync.dma_start(
                                out=g[:, ki * E:(ki + 1) * E], in_=gwtb[ki]
                            )
                        gwt_box.append(g)
                    gwt = gwt_box[0]
                    lp = lps.tile([E, NTOK], f32, tag="lp", name=f"lp_{s}")
                    for ch in range(4):
                        xr = xp.tile([128, 4 * NTOK], f32r, tag="xr", bufs=2,
                                     name=f"xr_{s}_{ch}")
                        for kj in range(4):
                            nc.sync.dma_start(
                                out=xr[:, kj * NTOK:(kj + 1) * NTOK],
                                in_=xrb[s, ch * 4 + kj],
                            )
                        for kj in range(4):
                            ki = ch * 4 + kj
                            nc.tensor.matmul(
                                lp[:],
                                gwt[:, ki * E:(ki + 1) * E],
                                xr[:, kj * NTOK:(kj + 1) * NTOK],
                                start=(ki == 0),
                                stop=(ki == KT_H - 1),
                            )
                    lsb = rp.tile([E, NTOK], f32, tag="lsb", bufs=1, name=f"lsb_{s}")
                    nc.vector.tensor_copy(lsb[:], lp[:])
                    nc.sync.dma_start(out=logb[:, s * NTOK:(s + 1) * NTOK], in_=lsb[:])
                return emit

            router_pending = [router_slice(s) for s in range(ns_r)]
            # slot-1 (K2) phases first; slot-0 last so its 256 tail (if any)
            # shortens the final drain. Non-final phases put the 256 tile
            # first for a faster post-boundary ramp.
            wb_, ob_ = _tiles(k2, tail_last=False)
            wa_, oa_ = _tiles(k1, tail_last=False)
            waL, oaL = _tiles(k1, tail_last=True)
            ffn_phase(0, 1, 0, k1, wb_, ob_)
            ffn_phase(1, 1, 1, k1, wb_, ob_)
            ffn_phase(2, 0, 0, 0, wa_, oa_)
            ffn_phase(3, 0, 1, 0, waL, oaL)
            while router_pending:
                router_pending.pop(0)()

    nc.compile()
    _BUILD_CACHE[key] = nc
    return nc


def kernel(hidden_states, gate_w, e_bias, w1, w2, w3):
    _ensure_ntff_hook()
    B, S, Hd = hidden_states.shape
    assert Hd == H
    x = np.ascontiguousarray(hidden_states.reshape(-1, H).astype(np.float32))
    T = x.shape[0]
    tslice = T // 8

    # ---- host routing (float64 for stable top-k decisions) ----
    logits64 = x.astype(np.float64) @ gate_w.T.astype(np.float64)
    scores = 1.0 / (1.0 + np.exp(-logits64))
    biased = scores + e_bias.astype(np.float64)
    sel = np.argsort(-biased, axis=1, kind="stable")[:, :TOP_K]       # [T, 2]
    rw = np.take_along_axis(scores, sel, axis=1)                       # [T, 2]
    rw = rw / np.maximum(rw.sum(-1, keepdims=True), 1e-12)
    rw = rw.astype(np.float32)

    idx = []
    wts = []
    for e in range(E):
        mask = sel == e                                                # [T, 2]
        ide = np.where(mask.any(1))[0]
        pos = mask[ide].argmax(1)
        idx.append(ide)
        wts.append(rw[ide, pos])

    # pair heaviest expert with lightest; each expert's tokens split over two
    # cores. Core 2i: first halves of pair i; core 2i+1: second halves.
    order = sorted(range(E), key=lambda e: -len(idx[e]))
    pairs = [(order[i], order[E - 1 - i]) for i in range(E // 2)]
    halves = {}
    for e in range(E):
        n = len(idx[e])
        u = (n + 1) // 2
        halves[e] = [(0, u), (u, n)]
    K1 = max(halves[a][0][1] for a, _ in pairs)
    K2 = max(halves[b][0][1] for _, b in pairs)
    K1 = max(TAIL, ((K1 + TAIL - 1) // TAIL) * TAIL)
    K2 = max(TAIL, ((K2 + TAIL - 1) // TAIL) * TAIL)
    R = K1 + K2

    nc = build(K1, K2, tslice)

    # ---- per-core inputs ----
    w1T = np.ascontiguousarray(np.transpose(w1, (0, 2, 1)).astype(np.float32))  # [E, H, I]
    w3T = np.ascontiguousarray(np.transpose(w3, (0, 2, 1)).astype(np.float32))  # [E, H, I]
    w2T = np.ascontiguousarray(np.transpose(w2, (0, 2, 1)).astype(np.float32))  # [E, I, H]
    gwtb = np.ascontiguousarray(gate_w.T.astype(np.float32).reshape(KT_H, 128, E))

    def wslices(e):
        a = w1T[e].reshape(KT_H, 128, 2, IH).transpose(2, 0, 1, 3)     # [2,KT_H,128,IH]
        b = w3T[e].reshape(KT_H, 128, 2, IH).transpose(2, 0, 1, 3)
        c2 = w2T[e].reshape(2, KT_I, 128, H)
        return a, b, c2

    wcache = {e: wslices(e) for e in range(E)}
    core_assign = []                                                   # per core: [(expert, lo, hi), (expert, lo, hi)]
    for (a, b) in pairs:
        for hi_ in range(2):
            core_assign.append([(a, *halves[a][hi_]), (b, *halves[b][hi_])])

    in_maps = []
    for c in range(E):
        (ea, lo_a, hi_a), (eb, lo_b, hi_b) = core_assign[c]
        xg = np.zeros((R, H), np.float32)
        cwf = np.zeros((R,), np.float32)
        na = hi_a - lo_a
        nb = hi_b - lo_b
        xg[:na] = x[idx[ea][lo_a:hi_a]]
        cwf[:na] = wts[ea][lo_a:hi_a]
        xg[K1:K1 + nb] = x[idx[eb][lo_b:hi_b]]
        cwf[K1:K1 + nb] = wts[eb][lo_b:hi_b]
        xtb = np.ascontiguousarray(xg.T)                               # [H, R]
        cwb = np.ascontiguousarray(np.broadcast_to(cwf, (128, R)))
        w1tb = np.ascontiguousarray(np.stack([wcache[ea][0], wcache[eb][0]]))
        w3tb = np.ascontiguousarray(np.stack([wcache[ea][1], wcache[eb][1]]))
        w2tb = np.ascontiguousarray(np.stack([wcache[ea][2], wcache[eb][2]]))
        xsl = x[c * tslice:(c + 1) * tslice]                           # [tslice, H]
        xrb = np.ascontiguousarray(
            xsl.reshape(tslice // NTOK, NTOK, H).transpose(0, 2, 1)
            .reshape(tslice // NTOK, KT_H, 128, NTOK)
        )
        in_maps.append(
            {
                "xtb": xtb,
                "cwb": cwb,
                "w1tb": w1tb,
                "w3tb": w3tb,
                "w2tb": w2tb,
                "xrb": xrb,
                "gwtb": gwtb,
            }
        )

    res = run_bass_kernel_spmd(nc, in_maps, list(range(8)))
    global LAST_RESULT
    LAST_RESULT = res

    # ---- combine on host ----
    out = np.zeros((T, H), np.float32)
    logits_out = np.empty((T, E), np.float32)
    for c in range(E):
        r = res.results[c]
        y = (r["yb"][0] + r["yb"][1]).T                                # [R, H]
        (ea, lo_a, hi_a), (eb, lo_b, hi_b) = core_assign[c]
        out[idx[ea][lo_a:hi_a]] += y[: hi_a - lo_a]
        out[idx[eb][lo_b:hi_b]] += y[K1:K1 + (hi_b - lo_b)]
        logits_out[c * tslice:(c + 1) * tslice] = r["logb"].T
    return out.reshape(B, S, H), logits_out


# revision 22
# speedup vs baseline: 1.0313x; 1.0313x over previous
"""MiniMax-M2 sparse MoE block on 8 Trainium2 NeuronCores.

Strategy: expert-parallel. Core c owns expert c's weights. The host computes
the routing (top-2 of 8, sigmoid scores + bias for selection) in float64,
gathers each expert's tokens, and ships them pre-transposed; each core runs
the gated FFN (silu(x@w1.T) * (x@w3.T)) @ w2.T over its gathered tokens in
float32r (full-rate fp32 matmul mode) and also computes the router logits for
its 1/8 slice of tokens. The host scatter-adds the weighted expert outputs.

Shapes (fixed by the problem): B=4, S=4096, H=2048, I=1024, E=8, top_k=2.

For load balance, experts are paired (heaviest with lightest) and each
expert's tokens are split across the pair's two cores, so every core runs
two expert "slots". The intermediate dim I is processed in two halves per
slot (4 weight phases total) so one phase's weights (~12.6 MB fp32) stay
resident in SBUF; each I-half emits a partial Y, summed on the host during
the un-permute. Router slices are interleaved between FFN tiles so their
matmuls run on a warm PE and their DMAs avoid the startup burst.
"""
import numpy as np

import concourse.mybir as mybir
from concourse import bacc
import concourse.tile as tile
from concourse.bass_utils import run_bass_kernel_spmd

H = 2048
I = 1024
E = 8
TOP_K = 2
KT_H = H // 128          # 16 k-tiles over H
IH = I // 2              # 512: half of intermediate dim
ISUB = IH // 128         # 4 psum subtiles per half
KT_I = IH // 128         # 4 k-tiles over I-half
NTOK = 512               # full token tile (matmul moving dim)
TAIL = 256               # token padding granularity (f32r full rate needs >=256)
f32 = mybir.dt.float32
f32r = mybir.dt.float32r
ACT_FN = mybir.ActivationFunctionType.Silu  # test_sim overrides (CoreSim lacks Silu)


def _ensure_ntff_hook():
    """If the environment requests tracing (BASS_TRACE) but lacks the
    antenv.axon_hooks module, inject it so profiling works instead of being
    silently skipped."""
    try:
        from antenv.axon_hooks import get_axon_ntff_profile_hook  # noqa: F401
        return
    except ImportError:
        pass
    try:
        import sys, types
        import antenv
        from trn_agent_boot.trn_boot import _ntff_profile_via_ctypes
        mod = types.ModuleType("antenv.axon_hooks")
        mod._hook = _ntff_profile_via_ctypes("/opt/axon/libaxon_pjrt.so")
        mod.set_axon_ntff_profile_hook = lambda h: setattr(mod, "_hook", h)
        mod.get_axon_ntff_profile_hook = lambda: mod._hook
        sys.modules["antenv.axon_hooks"] = mod
        antenv.axon_hooks = mod
    except Exception:
        pass


_BUILD_CACHE = {}
LAST_RESULT = None


def _tiles(n, tail_last):
    widths = [NTOK] * (n // NTOK)
    if n % NTOK:
        widths = widths + [TAIL] if tail_last else [TAIL] + widths
    offs = np.cumsum([0] + widths)[:-1].tolist()
    return widths, offs


def build(k1, k2, tslice):
    """Build the SPMD program: each core processes two expert slots with
    K1 / K2 gathered tokens (multiples of TAIL) and a router slice of
    `tslice` tokens."""
    key = (k1, k2, tslice)
    if key in _BUILD_CACHE:
        return _BUILD_CACHE[key]
    assert k1 % TAIL == 0 and k2 % TAIL == 0 and tslice % NTOK == 0
    ns_r = tslice // NTOK
    R = k1 + k2

    nc = bacc.Bacc("TRN2", target_bir_lowering=False, debug=False, num_devices=8)
    xtb = nc.dram_tensor("xtb", [H, R], f32r, kind="ExternalInput").ap()
    cwb = nc.dram_tensor("cwb", [128, R], f32, kind="ExternalInput").ap()
    w1tb = nc.dram_tensor("w1tb", [2, 2, KT_H, 128, IH], f32r, kind="ExternalInput").ap()
    w3tb = nc.dram_tensor("w3tb", [2, 2, KT_H, 128, IH], f32r, kind="ExternalInput").ap()
    w2tb = nc.dram_tensor("w2tb", [2, 2, KT_I, 128, H], f32r, kind="ExternalInput").ap()
    xrb = nc.dram_tensor("xrb", [ns_r, KT_H, 128, NTOK], f32r, kind="ExternalInput").ap()
    gwtb = nc.dram_tensor("gwtb", [KT_H, 128, E], f32r, kind="ExternalInput").ap()
    yb = nc.dram_tensor("yb", [2, H, R], f32, kind="ExternalOutput").ap()
    logb = nc.dram_tensor("logb", [E, tslice], f32, kind="ExternalOutput").ap()

    xtb_k = xtb.rearrange("(kt p) r -> kt p r", p=128)

    with tile.TileContext(nc) as tc:
        with (
            tc.tile_pool(name="wp", bufs=1) as wp,
            tc.tile_pool(name="xp", bufs=2) as xp,
            tc.tile_pool(name="pp", bufs=2) as pp,
            tc.tile_pool(name="cp", bufs=2) as cp,
            tc.tile_pool(name="yp", bufs=4) as yp,
            tc.tile_pool(name="rp", bufs=2) as rp,
            tc.tile_pool(name="gups", bufs=4, space="PSUM") as gups,
            tc.tile_pool(name="yps", bufs=3, space="PSUM") as yps,
            tc.tile_pool(name="lps", bufs=1, space="PSUM") as lps,
        ):
            def load_tile_inputs(ph, ti, off, w):
                xt = xp.tile([128, KT_H * w], f32r, tag="x", name=f"x_{ph}_{ti}")
                for ki in range(KT_H):
                    nc.sync.dma_start(
                        out=xt[:, ki * w:(ki + 1) * w],
                        in_=xtb_k[ki][:, off:off + w],
                    )
                cw = cp.tile([128, w], f32, tag="cw", bufs=1, name=f"cw_{ph}_{ti}")
                nc.sync.dma_start(out=cw[:], in_=cwb[:, off:off + w])
                return xt, cw

            def ffn_phase(ph, slot, h, base, widths, offs):
                # tile-0 inputs (and the first router slice) are queued BEFORE
                # the 12.6MB weight block so PE work starts within a few us.
                tile0 = load_tile_inputs(ph, 0, base + offs[0], widths[0])

                w1t, w3t, w2t = [], [], []
                for ki in range(KT_H):
                    t1 = wp.tile([128, IH], f32r, tag=f"w1_{ki}", name=f"w1_{ph}_{ki}")
                    nc.sync.dma_start(out=t1[:], in_=w1tb[slot, h, ki])
                    w1t.append(t1)
                    t3 = wp.tile([128, IH], f32r, tag=f"w3_{ki}", name=f"w3_{ph}_{ki}")
                    nc.sync.dma_start(out=t3[:], in_=w3tb[slot, h, ki])
                    w3t.append(t3)
                for ki in range(KT_I):
                    t2 = wp.tile([128, H], f32r, tag=f"w2_{ki}", name=f"w2_{ph}_{ki}")
                    nc.sync.dma_start(out=t2[:], in_=w2tb[slot, h, ki])
                    w2t.append(t2)

                for ti, (off0, w) in enumerate(zip(offs, widths)):
                    off = base + off0
                    if ti >= 1 and router_pending:
                        router_pending.pop(0)()
                    if ti == 0:
                        xt, cw = tile0
                    else:
                        xt, cw = load_tile_inputs(ph, ti, off, w)

                    pt = pp.tile([128, ISUB * w], f32r, tag="p", name=f"p_{ph}_{ti}")
                    for isub in range(ISUB):
                        gp = gups.tile([128, w], f32, tag="gu", name=f"g_{ph}_{ti}_{isub}")
                        up = gups.tile([128, w], f32, tag="gu", name=f"u_{ph}_{ti}_{isub}")
                        msl = slice(isub * 128, (isub + 1) * 128)
                        for ki in range(KT_H):
                            nc.tensor.matmul(
                                gp[:],
                                w1t[ki][:, msl],
                                xt[:, ki * w:(ki + 1) * w],
                                start=(ki == 0),
                                stop=(ki == KT_H - 1),
                            )
                        for ki in range(KT_H):
                            nc.tensor.matmul(
                                up[:],
                                w3t[ki][:, msl],
                                xt[:, ki * w:(ki + 1) * w],
                                start=(ki == 0),
                                stop=(ki == KT_H - 1),
                            )
                        psl = slice(isub * w, (isub + 1) * w)
                        sg = yp.tile([128, w], f32, tag="sg", bufs=1, name=f"sg_{ph}_{ti}_{isub}")
                        nc.scalar.activation(sg[:], gp[:], ACT_FN)
                        nc.vector.tensor_mul(pt[:, psl], up[:], sg[:])
                    for m in range(KT_H):
                        yps_t = yps.tile([128, w], f32, tag="y", name=f"y_{ph}_{ti}_{m}")
                        for ki in range(KT_I):
                            nc.tensor.matmul(
                                yps_t[:],
                                w2t[ki][:, m * 128:(m + 1) * 128],
                                pt[:, ki * w:(ki + 1) * w],
                                start=(ki == 0),
                                stop=(ki == KT_I - 1),
                            )
                        ysb = yp.tile([128, w], f32, tag="ysb", bufs=3, name=f"ysb_{ph}_{ti}_{m}")
                        nc.vector.tensor_mul(ysb[:], yps_t[:], cw[:])  # cw: per-token scale
                        nc.sync.dma_start(
                            out=yb[h][m * 128:(m + 1) * 128, off:off + w], in_=ysb[:]
                        )

            gwt_box = []

            def router_slice(s):
                def emit():
                    if not gwt_box:
                        g = rp.tile([128, KT_H * E], f32r, tag="gwt", bufs=1, name="gwt")
                        for ki in range(KT_H):
                            nc.sync.dma_start(
                                out=g[:, ki * E:(ki + 1) * E], in_=gwtb[ki]
                            )
                        gwt_box.append(g)
                    gwt = gwt_box[0]
                    lp = lps.tile([E, NTOK], f32, tag="lp", name=f"lp_{s}")
                    for ch in range(4):
                        xr = xp.tile([128, 4 * NTOK], f32r, tag="xr", bufs=2,
                                     name=f"xr_{s}_{ch}")
                        for kj in range(4):
                            nc.sync.dma_start(
                                out=xr[:, kj * NTOK:(kj + 1) * NTOK],
                                in_=xrb[s, ch * 4 + kj],
                            )
                        for kj in range(4):
                            ki = ch * 4 + kj
                            nc.tensor.matmul(
                                lp[:],
                                gwt[:, ki * E:(ki + 1) * E],
                                xr[:, kj * NTOK:(kj + 1) * NTOK],
                                start=(ki == 0),
                                stop=(ki == KT_H - 1),
                            )
                    lsb = rp.tile([E, NTOK], f32, tag="lsb", bufs=1, name=f"lsb_{s}")
                    nc.vector.tensor_copy(lsb[:], lp[:])
                    nc.sync.dma_start(out=logb[:, s * NTOK:(s + 1) * NTOK], in_=lsb[:])
                return emit

            router_pending = [router_slice(s) for s in range(ns_r)]
            # slot-1 (K2) phases first; slot-0 last so its 256 tail (if any)
            # shortens the final drain. Non-final phases put the 256 tile
            # first for a faster post-boundary ramp.
            wb_, ob_ = _tiles(k2, tail_last=False)
            wa_, oa_ = _tiles(k1, tail_last=False)
            waL, oaL = _tiles(k1, tail_last=True)
            ffn_phase(0, 1, 0, k1, wb_, ob_)
            ffn_phase(1, 1, 1, k1, wb_, ob_)
            ffn_phase(2, 0, 0, 0, wa_, oa_)
            ffn_phase(3, 0, 1, 0, waL, oaL)
            while router_pending:
                router_pending.pop(0)()

    nc.compile()
    _BUILD_CACHE[key] = nc
    return nc


def kernel(hidden_states, gate_w, e_bias, w1, w2, w3):
    _ensure_ntff_hook()
    hidden_states = np.asarray(hidden_states, np.float32)
    gate_w = np.asarray(gate_w, np.float32)
    e_bias = np.asarray(e_bias, np.float32)
    w1 = np.asarray(w1, np.float32)
    w2 = np.asarray(w2, np.float32)
    w3 = np.asarray(w3, np.float32)
    B, S, Hd = hidden_states.shape
    assert Hd == H
    x = np.ascontiguousarray(hidden_states.reshape(-1, H))
    T = x.shape[0]
    tslice = T // 8

    # ---- host routing (float64 for stable top-k decisions) ----
    logits64 = x.astype(np.float64) @ gate_w.T.astype(np.float64)
    scores = 1.0 / (1.0 + np.exp(-logits64))
    biased = scores + e_bias.astype(np.float64)
    sel = np.argsort(-biased, axis=1, kind="stable")[:, :TOP_K]       # [T, 2]
    rw = np.take_along_axis(scores, sel, axis=1)                       # [T, 2]
    rw = rw / np.maximum(rw.sum(-1, keepdims=True), 1e-12)
    rw = rw.astype(np.float32)

    idx = []
    wts = []
    for e in range(E):
        mask = sel == e                                                # [T, 2]
        ide = np.where(mask.any(1))[0]
        pos = mask[ide].argmax(1)
        idx.append(ide)
        wts.append(rw[ide, pos])

    # pair heaviest expert with lightest; each expert's tokens split over two
    # cores. Core 2i: first halves of pair i; core 2i+1: second halves.
    order = sorted(range(E), key=lambda e: -len(idx[e]))
    pairs = [(order[i], order[E - 1 - i]) for i in range(E // 2)]
    halves = {}
    for e in range(E):
        n = len(idx[e])
        u = (n + 1) // 2
        halves[e] = [(0, u), (u, n)]
    K1 = max(halves[a][0][1] for a, _ in pairs)
    K2 = max(halves[b][0][1] for _, b in pairs)
    K1 = max(TAIL, ((K1 + TAIL - 1) // TAIL) * TAIL)
    K2 = max(TAIL, ((K2 + TAIL - 1) // TAIL) * TAIL)
    R = K1 + K2

    nc = build(K1, K2, tslice)

    # ---- per-core inputs ----
    w1T = np.ascontiguousarray(np.transpose(w1, (0, 2, 1)).astype(np.float32))  # [E, H, I]
    w3T = np.ascontiguousarray(np.transpose(w3, (0, 2, 1)).astype(np.float32))  # [E, H, I]
    w2T = np.ascontiguousarray(np.transpose(w2, (0, 2, 1)).astype(np.float32))  # [E, I, H]
    gwtb = np.ascontiguousarray(gate_w.T.astype(np.float32).reshape(KT_H, 128, E))

    def wslices(e):
        a = w1T[e].reshape(KT_H, 128, 2, IH).transpose(2, 0, 1, 3)     # [2,KT_H,128,IH]
        b = w3T[e].reshape(KT_H, 128, 2, IH).transpose(2, 0, 1, 3)
        c2 = w2T[e].reshape(2, KT_I, 128, H)
        return a, b, c2

    wcache = {e: wslices(e) for e in range(E)}
    core_assign = []                                                   # per core: [(expert, lo, hi), (expert, lo, hi)]
    for (a, b) in pairs:
        for hi_ in range(2):
            core_assign.append([(a, *halves[a][hi_]), (b, *halves[b][hi_])])

    in_maps = []
    for c in range(E):
        (ea, lo_a, hi_a), (eb, lo_b, hi_b) = core_assign[c]
        xg = np.zeros((R, H), np.float32)
        cwf = np.zeros((R,), np.float32)
        na = hi_a - lo_a
        nb = hi_b - lo_b
        xg[:na] = x[idx[ea][lo_a:hi_a]]
        cwf[:na] = wts[ea][lo_a:hi_a]
        xg[K1:K1 + nb] = x[idx[eb][lo_b:hi_b]]
        cwf[K1:K1 + nb] = wts[eb][lo_b:hi_b]
        xtb = np.ascontiguousarray(xg.T)                               # [H, R]
        cwb = np.ascontiguousarray(np.broadcast_to(cwf, (128, R)))
        w1tb = np.ascontiguousarray(np.stack([wcache[ea][0], wcache[eb][0]]))
        w3tb = np.ascontiguousarray(np.stack([wcache[ea][1], wcache[eb][1]]))
        w2tb = np.ascontiguousarray(np.stack([wcache[ea][2], wcache[eb][2]]))
        xsl = x[c * tslice:(c + 1) * tslice]                           # [tslice, H]
        xrb = np.ascontiguousarray(
            xsl.reshape(tslice // NTOK, NTOK, H).transpose(0, 2, 1)
            .reshape(tslice // NTOK, KT_H, 128, NTOK)
        )
        in_maps.append(
            {
                "xtb": xtb,
                "cwb": cwb,
                "w1tb": w1tb,
                "w3tb": w3tb,
                "w2tb": w2tb,
                "xrb": xrb,
                "gwtb": gwtb,
            }
        )

    res = run_bass_kernel_spmd(nc, in_maps, list(range(8)))
    global LAST_RESULT
    LAST_RESULT = res

    # ---- combine on host ----
    out = np.zeros((T, H), np.float32)
    logits_out = np.empty((T, E), np.float32)
    for c in range(E):
        r = res.results[c]
        y = (r["yb"][0] + r["yb"][1]).T                                # [R, H]
        (ea, lo_a, hi_a), (eb, lo_b, hi_b) = core_assign[c]
        out[idx[ea][lo_a:hi_a]] += y[: hi_a - lo_a]
        out[idx[eb][lo_b:hi_b]] += y[K1:K1 + (hi_b - lo_b)]
        logits_out[c * tslice:(c + 1) * tslice] = r["logb"].T
    return out.reshape(B, S, H), logits_out
